# revision 7
# baseline (speedup 1.0000x reference)
"""Trainium2 Bass kernel for ConditionalLogisticRegression (segment softmax).

Computes out = segment_softmax(X @ W + b, segment_ids) over 8 NeuronCores.

Strategy:
- Shard rows across 8 cores at stratum boundaries (np.searchsorted on the
  sorted segment_ids), replicate tiny W/b. Each core gets a zero-padded
  fixed-size shard.
- Phase A (per core): stream X in natural row-major tiles [128, 128*64],
  logits = rowwise dot with W via DVE multiply + free-dim reduce, write
  logits to a DRAM scratch.
- Phase B (per core): reload logits + segment ids in a partition-major
  layout with halos (partition p owns rows [p*R, p*R+R+2H)), then do the
  entire ragged segment softmax with two tensor_tensor_scan ops:
    m[t]   = (seg[t] == seg[t-1])            # segment-continuation mask
    C[t]   = m[t]*C[t-1] + e[t]              # segmented inclusive cumsum
    T[t]   = max(m[t+1]*T[t+1], C[t])        # backward total-propagate
    out[t] = e[t] / T[t]
  Halos of H >= max segment length make each partition independent.
"""

import sys

sys.path.insert(0, "/opt/trn_rl_repo")

import numpy as np

import concourse.bass as bass
import concourse.mybir as mybir
from concourse.bass_utils import run_bass_kernel_spmd

# ---- problem constants (hardcoded per contest contract) ----
N = 4194304
D = 64
NUM_SEG = 131072
N_CORES = 8

# ---- kernel layout constants ----
H = 128            # halo (must exceed max segment length; seed-0 max is ~60)
R = 4224           # rows per partition in phase B
MAXR = 128 * R     # padded shard size = 540672
CHUNK = 16384      # rows per phase-A chunk ([128, 128*64] tile)
N_CHUNKS = 34
P_TOT = N_CHUNKS * CHUNK  # 557056 padded rows incl. halo slop
WIN = R + 2 * H    # phase-B per-partition window = 4480

f32 = mybir.dt.float32
i32 = mybir.dt.int32
Alu = mybir.AluOpType
Act = mybir.ActivationFunctionType

_PROGRAM = None
_LAST_RESULTS = None


def _build_program():
    nc = bass.Bass()

    x_ext = nc.declare_dram_parameter("x", [P_TOT, D], f32, isOutput=False)
    seg_ext = nc.declare_dram_parameter("seg", [P_TOT], i32, isOutput=False)
    w_ext = nc.declare_dram_parameter("w", [128, D], f32, isOutput=False)
    b_ext = nc.declare_dram_parameter("bb", [128, 1], f32, isOutput=False)
    out_ext = nc.declare_dram_parameter("out", [MAXR], f32, isOutput=True)
    lg_dram = nc.dram_tensor("logits_scratch", [P_TOT], f32)

    with (
        nc.sbuf_tensor([128, CHUNK // 128 * D], f32) as xt0,
        nc.sbuf_tensor([128, CHUNK // 128 * D], f32) as xt1,
        nc.sbuf_tensor([128, CHUNK // 128], f32) as lg0,
        nc.sbuf_tensor([128, CHUNK // 128], f32) as lg1,
        nc.sbuf_tensor([128, D], f32) as wt,
        nc.sbuf_tensor([128, 1], f32) as bt,
        nc.sbuf_tensor([128, WIN], i32) as segt,
        nc.sbuf_tensor([128, WIN], f32) as segf,
        nc.sbuf_tensor([128, WIN + 4], f32) as mt,
        nc.sbuf_tensor([128, WIN], f32) as lgt,
        nc.sbuf_tensor([128, WIN], f32) as et,
        nc.sbuf_tensor([128, WIN], f32) as Ct,
        nc.sbuf_tensor([128, WIN], f32) as Tt,
        nc.semaphore("dma_in") as dma_in,
        nc.semaphore("dma_out") as dma_out,
        nc.semaphore("v_sem") as v_sem,
        nc.semaphore("dma_in_b") as dma_in_b,
        nc.semaphore("a_b") as a_b,
        nc.semaphore("v_b") as v_b,
        nc.semaphore("dma_out_b") as dma_out_b,
        nc.Block() as block,
    ):
        xts = [xt0, xt1]
        lgs = [lg0, lg1]
        FPP = CHUNK // 128  # rows per partition per chunk = 128

        def x_chunk_ap(c):
            return bass.AP(
                tensor=x_ext[:].tensor,
                offset=c * CHUNK * D,
                ap=[[FPP * D, 128], [1, FPP * D]],
            )

        def lg_chunk_ap(c):
            return bass.AP(
                tensor=lg_dram[:].tensor,
                offset=c * CHUNK,
                ap=[[FPP, 128], [1, FPP]],
            )

        lg_win_ap = bass.AP(
            tensor=lg_dram[:].tensor, offset=0, ap=[[R, 128], [1, WIN]]
        )
        seg_win_ap = bass.AP(
            tensor=seg_ext[:].tensor, offset=0, ap=[[R, 128], [1, WIN]]
        )
        out_ap = bass.AP(
            tensor=out_ext[:].tensor, offset=0, ap=[[R, 128], [1, R]]
        )

        @block.sync
        def _(sync):
            sync.dma_start(wt[:], w_ext[:]).then_inc(dma_in, 16)
            sync.dma_start(bt[:], b_ext[:]).then_inc(dma_in, 16)
            for c in range(N_CHUNKS):
                if c >= 2:
                    sync.wait_ge(v_sem, c - 1)  # xt[c%2] free
                sync.dma_start(xts[c % 2][:], x_chunk_ap(c)).then_inc(dma_in, 16)
                if c >= 1:
                    sync.wait_ge(v_sem, c)  # lg[(c-1)%2] ready
                    sync.dma_start(lg_chunk_ap(c - 1), lgs[(c - 1) % 2][:]).then_inc(
                        dma_out, 16
                    )
            sync.wait_ge(v_sem, N_CHUNKS)
            sync.dma_start(lg_chunk_ap(N_CHUNKS - 1), lgs[(N_CHUNKS - 1) % 2][:]).then_inc(
                dma_out, 16
            )
            # ---- phase B ----
            sync.wait_ge(dma_out, 16 * N_CHUNKS)  # all logits landed in DRAM
            sync.dma_start(segt[:], seg_win_ap).then_inc(dma_in_b, 16)
            sync.dma_start(lgt[:], lg_win_ap).then_inc(dma_in_b, 16)
            sync.wait_ge(v_b, 1)
            sync.dma_start(out_ap, et[:, H : H + R]).then_inc(dma_out_b, 16)
            sync.wait_ge(dma_out_b, 16)

        @block.vector
        def _(vector):
            for c in range(N_CHUNKS):
                vector.wait_ge(dma_in, 32 + 16 * (c + 1))
                x3 = xts[c % 2][:].rearrange("p (r f) -> p r f", f=D)
                w3 = wt[:].unsqueeze(1).broadcast_to([128, FPP, D])
                nc.vector.tensor_tensor(x3, x3, w3, Alu.mult)
                if c >= 2:
                    vector.wait_ge(dma_out, 16 * (c - 1))  # lg[c%2] drained
                nc.vector.tensor_reduce(
                    lgs[c % 2][:], x3, axis=mybir.AxisListType.X, op=Alu.add
                ).then_inc(v_sem, 1)
            # ---- phase B ----
            vector.memset(mt[:, 0:1], 0.0)
            vector.memset(mt[:, WIN : WIN + 1], 0.0)
            vector.wait_ge(dma_in_b, 16)  # segt
            nc.vector.tensor_copy(segf[:], segt[:])  # exact int32 -> fp32 cast
            nc.vector.tensor_tensor(
                mt[:, 1:WIN], segf[:, 1:WIN], segf[:, 0 : WIN - 1], Alu.is_equal
            )
            vector.wait_ge(a_b, 1)  # e ready
            nc.vector.tensor_tensor_scan(
                Ct[:], mt[:, 0:WIN], et[:], 0.0, op0=Alu.mult, op1=Alu.add
            )
            nc.vector.tensor_tensor_scan(
                Tt[:, ::-1],
                mt[:, WIN:0:-1],
                Ct[:, ::-1],
                0.0,
                op0=Alu.mult,
                op1=Alu.max,
            )
            nc.vector.reciprocal(Tt[:], Tt[:])
            nc.vector.tensor_tensor(et[:], et[:], Tt[:], Alu.mult).then_inc(v_b, 1)

        @block.scalar
        def _(scalar):
            scalar.wait_ge(dma_in, 32)  # wt, bt
            scalar.wait_ge(dma_in_b, 32)  # segt + lgt
            nc.scalar.activation(
                et[:], lgt[:], Act.Exp, bias=bt[:], scale=1.0
            ).then_inc(a_b, 1)

    return nc


def _get_program():
    global _PROGRAM
    if _PROGRAM is None:
        _PROGRAM = _build_program()
    return _PROGRAM


def kernel(X, segment_ids, W, b):
    X = np.asarray(X, dtype=np.float32)
    segment_ids = np.asarray(segment_ids, dtype=np.int32)
    W = np.asarray(W, dtype=np.float32)
    b = np.asarray(b, dtype=np.float32)

    n = X.shape[0]
    seg_per_core = NUM_SEG // N_CORES
    bounds = [
        int(np.searchsorted(segment_ids, m * seg_per_core, side="left"))
        for m in range(N_CORES + 1)
    ]
    bounds[0], bounds[-1] = 0, n

    # sanity: halo must exceed the longest stratum
    max_run = int(np.max(np.diff(np.flatnonzero(np.diff(segment_ids) != 0)))) \
        if n > 1 else 1
    assert max_run < H, f"segment run {max_run} >= halo {H}"

    in_maps = []
    w_rep = np.tile(W[None, :], (128, 1)).astype(np.float32)
    b_rep = np.full((128, 1), float(b[0]), dtype=np.float32)
    fake_ids = (NUM_SEG + 16 + (np.arange(P_TOT, dtype=np.int64) // 16)).astype(
        np.int32
    )
    for m in range(N_CORES):
        lo, hi = bounds[m], bounds[m + 1]
        size = hi - lo
        assert size <= MAXR, f"shard {m} size {size} > MAXR {MAXR}"
        xp = np.zeros((P_TOT, D), dtype=np.float32)
        xp[H : H + size] = X[lo:hi]
        sp = fake_ids.copy()
        sp[H : H + size] = segment_ids[lo:hi]
        in_maps.append({"x": xp, "seg": sp, "w": w_rep, "bb": b_rep})

    nc = _get_program()
    res = run_bass_kernel_spmd(nc, in_maps, list(range(N_CORES)))
    global _LAST_RESULTS
    _LAST_RESULTS = res

    out = np.empty(n, dtype=np.float32)
    for m in range(N_CORES):
        lo, hi = bounds[m], bounds[m + 1]
        out[lo:hi] = res.results[m]["out"][: hi - lo]
    return out
